# revision 22
# baseline (speedup 1.0000x reference)
"""Trainium2 Bass kernel for nn_DPP: batched masked-Gram logdet minus shared
normalizer logdet.

out[i] = logdet(G * m_i m_i^T + diag(1-m_i)) - logdet(G + I),  G = B^T B

Sharding: data-parallel over the batch dim of x (one sample per NeuronCore).
Host-side trick: each core receives B with its sample's SELECTED columns
permuted to the front.  Then ONE Gram G' = Bperm^T Bperm serves both
factorizations:
  - masked matrix = leading [1152 x 1152] block of G' with a contiguous
    prefix mask (nsel <= 1058 < 1152 for this problem) -> 9-panel Cholesky
    instead of 16 (the trailing 896+ masked cols are identity rows, det 1).
  - normalizer  = G' + I (full 2048, det invariant under permutation)
    -> 16-panel Cholesky.
Each core computes the shared logdet(G+I) redundantly (no cross-core
traffic; collectives here cost more than the 4.5 MB recompute).

Device algorithm (per core):
  - G' upper-triangle strips via bf16 matmuls (fp32 PSUM accum), emitted
    interleaved with the Cholesky panels so PE overlaps both.
  - Two interleaved left-looking blocked Cholesky factorizations (U-form,
    128-wide panels): A = leading window masked (9 panels), B = G'+I (16
    panels).  B panels 0-6 run solo first (their big Schur updates + gram
    strips are PE filler), then (B_{7+i}, A_i) zip so both refine chains
    overlap; trailing widths shrink together.
  - Each 128x128 diagonal pivot S is handled matmul-only ("refine" scheme):
      d = diag(S); r = 1/sqrt(d)                  (DVE reciprocal + ACT Sqrt)
      corr = S * (r r^T); X1 = striu(corr); X1T = stril(corr)
      W = diag(r) (I - X1 + X1@X1)                (approx inv-chol factor)
      F = W^T S W - I                             (small: ||F|| ~ 0.15)
      logdet(S) = sum(ln d) + tr F - tr F^2/2 + tr F^3/3
      What = W + W(-F/2 + 3F^2/8)                 (What What^T ~ S^{-1} to O(F^3))
    Panel: U_strip = What^T @ strip; trailing Schur updates use U (bf16).
    All ln d are batched into one ACT Ln at the end (2 table loads total).
"""

import numpy as np
import ml_dtypes

P = 128
N = 2048           # full matrix dim (= n columns of B)
NTB = 16           # panels of the normalizer factorization
NTA = 9            # panels of the masked factorization (window 1152)
NS = NTA * P       # masked window = 1152 cols
NKT = 16           # contraction tiles (B rows padded 2000 -> 2048)
FT = 512           # free-dim tile for wide matmuls

_CACHE = {}


def _col_tiles(width_blocks, base_col, diag_first=False):
    """Split absolute cols [base_col, base_col + width_blocks*128) into <=512
    tiles. With diag_first, the first tile is exactly 128 wide (diag block)."""
    tiles = []
    c = base_col
    end = base_col + width_blocks * P
    if diag_first:
        tiles.append((c, P))
        c += P
    while c < end:
        w = min(FT, end - c)
        tiles.append((c, w))
        c += w
    return tiles


def _build():
    import concourse.bass as bass
    import concourse.bacc as bacc
    import concourse.mybir as mybir
    from concourse.bass import ds, ts
    from concourse.masks import (
        make_identity,
        make_upper_triangular,
        make_lower_triangular,
    )
    from concourse.tile import TileContext
    from contextlib import ExitStack

    f32 = mybir.dt.float32
    bf16 = mybir.dt.bfloat16
    f8 = mybir.dt.float8e4
    DR = mybir.MatmulPerfMode.DoubleRow
    AF = mybir.ActivationFunctionType
    OP = mybir.AluOpType
    PSUM = bass.MemorySpace.PSUM
    AX = mybir.AxisListType.X

    NPAN = NTA + NTB  # 25 total panels

    nc = bacc.Bacc()
    bb = nc.dram_tensor("bb", [N, N], f8, kind="ExternalInput")
    mrow_d = nc.dram_tensor("mrow", [1, NS], bf16, kind="ExternalInput")
    mcol_d = nc.dram_tensor("mcol", [NS, 1], f32, kind="ExternalInput")
    out_d = nc.dram_tensor("out", [1, 1], f32, kind="ExternalOutput")

    with TileContext(nc) as tc, ExitStack() as stack:
        consts = stack.enter_context(tc.tile_pool(name="consts", bufs=1))
        I128 = consts.tile([P, P], f32, tag="i128")
        make_identity(nc, I128)
        I128b = consts.tile([P, P], bf16, tag="i128b")
        nc.vector.tensor_copy(I128b, I128)
        STRIU = consts.tile([P, P], f32, tag="striu")
        make_upper_triangular(nc, STRIU, val=1.0, diag=False)
        STRIUB = consts.tile([P, P], bf16, tag="striub")
        nc.vector.tensor_copy(STRIUB, STRIU)
        mrow = consts.tile([1, NS], bf16, tag="mrow")
        nc.sync.dma_start(mrow, mrow_d[:, :])
        mcol = consts.tile([P, NTA], f32, tag="mcol")
        nc.sync.dma_start(mcol, mcol_d.rearrange("(t p) one -> p (t one)", p=P))
        acc = consts.tile([P, 2], f32, tag="acc")
        nc.vector.memset(acc, 0.0)
        dstore = consts.tile([P, NPAN], f32, tag="dstore")
        # B is fed as fp8 scaled by 16, so the Gram is 256*G; diag fixes are
        # scaled by 256 to match and the host adds (N-NS)*ln(256) back.
        SC = 256.0
        onem_all = consts.tile([P, NTA], f32, tag="onem_all")
        nc.vector.tensor_scalar(
            out=onem_all, in0=mcol, scalar1=-SC, scalar2=SC,
            op0=OP.mult, op1=OP.add,
        )
        I256 = consts.tile([P, P], f32, tag="i256")
        nc.vector.tensor_scalar(
            out=I256, in0=I128, scalar1=SC, scalar2=None, op0=OP.mult
        )
        # diag fix for masked panels: SC*diag(1-m) per 128-block
        dfix_all = consts.tile([P, NTA, P], f32, tag="dfix_all")
        for i in range(NTA):
            nc.vector.tensor_scalar_mul(dfix_all[:, i, :], I128, onem_all[:, ds(i, 1)])

        # shared Gram strips: gs[i]: [P, (NTB-i)*P] bf16, cols i*128..2048
        gs = []
        for i in range(NTB):
            gs.append(consts.tile([P, (NTB - i) * P], bf16, tag=f"gs{i}", name=f"gs{i}"))
        # U panels, fp8, one tensor per factorization with ABSOLUTE columns:
        # ubig[m][:, j, c] = U_j[:, c].  Uniform panel stride lets the Schur
        # chains pair two panels into one DoubleRow (double-pumped) matmul.
        # fp8 U storage costs ~0.1 abs logdet error (CPU-simulated; budget 30).
        ubig = {
            0: consts.tile([P, NTA, NTA * P], f8, tag="ubigA", name="ubigA"),
            1: consts.tile([P, NTB, NTB * P], f8, tag="ubigB", name="ubigB"),
        }

        NT_of = {0: NTA, 1: NTB}

        bpool = stack.enter_context(tc.tile_pool(name="bpool", bufs=1))
        gpsum = stack.enter_context(tc.tile_pool(name="gram_psum", bufs=2, space=PSUM))
        spool = stack.enter_context(tc.tile_pool(name="strip_pool", bufs=2))
        rpool = stack.enter_context(tc.tile_pool(name="ref_pool", bufs=2))
        vpool = stack.enter_context(tc.tile_pool(name="vec_pool", bufs=2))
        apsum = stack.enter_context(tc.tile_pool(name="acc_psum", bufs=2, space=PSUM))
        wpsum = stack.enter_context(tc.tile_pool(name="work_psum", bufs=4, space=PSUM))

        bt = bpool.tile([P, NKT, N], f8, tag="bt")
        # per-ktile DMAs so the first Gram chains can start before the full
        # 4.2 MB lands (a single DMA serialized ~35 us of startup)
        for kt in range(NKT):
            nc.sync.dma_start(bt[:, kt, :], bb[ds(kt * P, P), :])

        def gram_chunks(i):
            """One yield per <=512-wide tile of Gram strip i (8 double-pumped
            fp8 MMs, 256-deep contraction each)."""
            for (c0, w) in _col_tiles(NTB - i, i * P):
                pt = gpsum.tile([P, FT], f32, tag="gp", name="pt")
                for kt in range(NKT // 2):
                    nc.tensor.matmul(
                        pt[:, :w],
                        bt[:, ds(2 * kt, 2), ts(i, P)],
                        bt[:, ds(2 * kt, 2), ds(c0, w)],
                        start=(kt == 0),
                        stop=(kt == NKT // 2 - 1),
                        perf_mode=DR,
                    )
                nc.scalar.copy(gs[i][:, ds(c0 - i * P, w)], pt[:, :w])
                yield

        def new_panel(i, m):
            wblk = NT_of[m] - i
            return {
                "m": m,
                "i": i,
                "tiles": _col_tiles(wblk, i * P, diag_first=True),
                # tiles that outlive the group (read by deferred TRSM/traces
                # emitted during the NEXT group) get per-matrix tags so the
                # bufs=2 rotation can't clobber them early.
                "strip": spool.tile(
                    [P, wblk * P], bf16, tag=f"strip{m}", name="strip"
                ),
                "sblk": rpool.tile([P, P], f32, tag="sblk", name="sblk"),
                "sb": rpool.tile([P, P], bf16, tag="sb", name="sb"),
                # dstore column and acc column for this panel
                "dcol": i if m == 0 else NTA + i,
                "acol": m,
            }

        def emit_ap_chain(m, i, c0, w):
            """Schur accumulator for cols [c0, c0+w): pairs of fp8 U panels
            via double-pumped matmuls."""
            ap = apsum.tile([P, FT], f32, tag="ap", name="ap")
            npair = i // 2
            for jp in range(npair):
                nc.tensor.matmul(
                    ap[:, :w],
                    ubig[m][:, ds(2 * jp, 2), ds(i * P, P)],
                    ubig[m][:, ds(2 * jp, 2), ds(c0, w)],
                    start=(jp == 0),
                    stop=(jp == npair - 1 and i % 2 == 0),
                    perf_mode=DR,
                )
            if i % 2 == 1:
                nc.tensor.matmul(
                    ap[:, :w],
                    ubig[m][:, i - 1, ds(i * P, P)],
                    ubig[m][:, i - 1, ds(c0, w)],
                    start=(i == 1),
                    stop=True,
                )
            return ap

        def emit_diag_ap(cx):
            """PE phase of the diag-tile prep (emitted first so deferred
            trace/DVE work can overlap it on other engines)."""
            i, m = cx["i"], cx["m"]
            if i > 0:
                cx["diag_ap"] = emit_ap_chain(m, i, i * P, P)

        def emit_diag_fin(cx):
            """DVE phase of the diag-tile prep."""
            i, m = cx["i"], cx["m"]
            sblk, sb = cx["sblk"], cx["sb"]
            ap = cx.get("diag_ap")
            gsl = gs[i][:, ds(0, P)]
            if m == 0:
                mo = wpsum.tile([P, FT], f32, tag="w", name="mo")
                nc.tensor.matmul(
                    mo[:, :P], mrow[:, ts(i, P)], mrow[:, ts(i, P)],
                    start=True, stop=True,
                )
                tmp = rpool.tile([P, P], f32, tag="tmp", name="tmp")
                nc.vector.tensor_mul(tmp, gsl, mo[:, :P])
                if i > 0:
                    tmp2 = rpool.tile([P, P], f32, tag="tmp2", name="tmp2")
                    nc.vector.tensor_sub(tmp2, tmp, ap[:, :P])
                else:
                    tmp2 = tmp
                nc.vector.tensor_add(sblk, tmp2, dfix_all[:, i, :])
            else:
                if i > 0:
                    tmp = rpool.tile([P, P], f32, tag="tmp", name="tmp")
                    nc.vector.tensor_sub(tmp, gsl, ap[:, :P])
                    nc.vector.tensor_add(sblk, tmp, I256)
                else:
                    nc.vector.tensor_add(sblk, gsl, I256)
            nc.vector.tensor_copy(sb, sblk)

        def emit_accum_prep(i, m, cx, tix):
            """Accum psum chain + strip-prep for OFF-DIAG tile tix (>0)."""
            c0, w = cx["tiles"][tix]
            strip = cx["strip"]
            ap = None
            if i > 0:
                ap = emit_ap_chain(m, i, c0, w)
            gsl = gs[i][:, ds(c0 - i * P, w)]
            if m == 0:
                # masked window: strip = gs * (m m^T) [- ap]
                mo = wpsum.tile([P, FT], f32, tag="w", name="mo")
                nc.tensor.matmul(
                    mo[:, :w], mrow[:, ts(i, P)], mrow[:, ds(c0, w)],
                    start=True, stop=True,
                )
                tmp3 = spool.tile([P, FT], f32, tag="ptmp", name="tmp3")
                nc.vector.tensor_mul(tmp3[:, :w], gsl, mo[:, :w])
                if i > 0:
                    nc.vector.tensor_sub(
                        strip[:, ds(c0 - i * P, w)], tmp3[:, :w], ap[:, :w]
                    )
                else:
                    nc.vector.tensor_copy(
                        strip[:, ds(c0 - i * P, w)], tmp3[:, :w]
                    )
            else:
                if i > 0:
                    nc.vector.tensor_sub(
                        strip[:, ds(c0 - i * P, w)], gsl, ap[:, :w]
                    )
                # (m=1, i=0): TRSM reads gs[0] directly

        def refine_gen(cx):
            """Pivot-block factor; yields at cross-engine handoffs so filler
            matmuls can be emitted between dependent steps.  W = diag(r)(I-X1)
            first-order with one -F/2 refinement; F+I = W'^T c1 W' is computed
            straight from the normalized pivot c1 (c1 = diag(r) S diag(r)), so
            the W-scaling (wfac) and its transpose (wt) hang OFF the critical
            chain.  Trace series (to F^2) deferred into trace_chunks()."""
            m = cx["m"]
            sblk = cx["sblk"]
            dcol = dstore[:, ds(cx["dcol"], 1)]
            dummy = rpool.tile([P, P], f32, tag="dummy", name="dummy")
            nc.vector.tensor_mul(dummy, sblk, I128)
            nc.vector.tensor_reduce(dcol, dummy, AX, OP.add)
            rinv = vpool.tile([P, 1], f32, tag="rinv", name="rinv")
            nc.vector.reciprocal(rinv, dcol)
            rcol = vpool.tile([P, 1], f32, tag="rcol", name="rcol")
            nc.scalar.sqrt(rcol, rinv)
            yield
            rt_ps = wpsum.tile([P, FT], f32, tag="w", name="rt_ps")
            nc.tensor.transpose(rt_ps[:1, :P], rcol, I128)
            rrow = vpool.tile([1, P], bf16, tag="rrow", name="rrow")
            nc.vector.tensor_copy(rrow, rt_ps[:1, :P])
            yield
            q_ps = wpsum.tile([P, FT], f32, tag="w", name="q_ps")
            nc.tensor.matmul(q_ps[:, :P], rrow, rrow, start=True, stop=True)
            c1b = rpool.tile([P, P], bf16, tag="c1b", name="c1b")
            nc.vector.tensor_mul(c1b, sblk, q_ps[:, :P])
            yield
            x1 = rpool.tile([P, P], bf16, tag="x1", name="x1")
            nc.gpsimd.tensor_mul(x1, c1b, STRIUB)
            wser = rpool.tile([P, P], bf16, tag="wser", name="wser")
            nc.vector.tensor_sub(wser, I128b, x1)
            yield
            y_ps = wpsum.tile([P, FT], f32, tag="w", name="y_ps")
            nc.tensor.matmul(y_ps[:, :P], c1b, wser, start=True, stop=True)
            yb = rpool.tile([P, P], bf16, tag="yb", name="yb")
            nc.vector.tensor_copy(yb, y_ps[:, :P])
            # off-chain: wfac = diag(r) (I - X1)
            wfac = rpool.tile([P, P], bf16, tag="wfac", name="wfac")
            nc.vector.tensor_scalar_mul(wfac, wser, rcol)
            yield
            f_ps = wpsum.tile([P, FT], f32, tag="w", name="f_ps")
            nc.tensor.matmul(f_ps[:, :P], wser, yb, start=True, stop=True)
            ff = rpool.tile([P, P], bf16, tag=f"ff{m}", name="ff")
            nc.vector.tensor_sub(ff, f_ps[:, :P], I128)
            fs = rpool.tile([P, P], bf16, tag="fs", name="fs")
            nc.vector.tensor_scalar_mul(fs, ff, -0.5)
            yield
            # off-chain: wt = wfac^T (ready before wh thanks to the F chain)
            wt_ps = wpsum.tile([P, FT * 2], bf16, tag="w", name="wt_ps")
            nc.tensor.transpose(wt_ps[:, :P], wfac, I128b)
            wt = rpool.tile([P, P], bf16, tag="wt", name="wt")
            nc.vector.tensor_copy(wt, wt_ps[:, :P])
            yield
            wh_ps = wpsum.tile([P, FT], f32, tag="w", name="wh_ps")
            nc.tensor.matmul(wh_ps[:, :P], wt, fs, start=True, stop=True)
            what = rpool.tile([P, P], bf16, tag=f"what{m}", name="what")
            nc.vector.tensor_add(what, wh_ps[:, :P], wfac)
            cx["what"] = what
            cx["ff"] = ff

        def trace_chunks(cx):
            """Deferred logdet trace series (to F^2) for a finished panel:
            emitted as filler in the NEXT round so it never sits in an engine
            queue ahead of the refine chain's dependent ops."""
            ff = cx["ff"]
            dummy3 = rpool.tile([P, P], f32, tag="dummy3", name="dummy3")
            nc.gpsimd.tensor_mul(dummy3, ff, I128)
            trf = vpool.tile([P, 1], f32, tag="trf", name="trf")
            nc.vector.tensor_reduce(trf, dummy3, AX, OP.add)
            yield
            dummy4 = rpool.tile([P, P], f32, tag="dummy4", name="dummy4")
            nc.gpsimd.tensor_mul(dummy4, ff, ff)
            trf2 = vpool.tile([P, 1], f32, tag="trf2", name="trf2")
            nc.vector.tensor_reduce(trf2, dummy4, AX, OP.add)
            yield
            t1 = vpool.tile([P, 1], f32, tag="t1", name="t1")
            t2 = vpool.tile([P, 1], f32, tag="t2", name="t2")
            nc.vector.tensor_scalar(
                out=t2, in0=trf2, scalar1=-0.5, scalar2=None, op0=OP.mult
            )
            nc.vector.tensor_add(t1, trf, t2)
            ac = cx["acol"]
            nc.vector.tensor_add(acc[:, ds(ac, 1)], acc[:, ds(ac, 1)], t1)

        def _trsm_tiles(cx):
            """TRSM tiling: diag, one 128 block, then <=512 chunks.  The
            first two are emitted in-round (the next diag-prep needs U's
            col-block 1); the rest defers into the next round as PE filler."""
            i, m = cx["i"], cx["m"]
            base, end = i * P, NT_of[m] * P
            tiles = [(base, P)]
            c = base + P
            if c < end:
                tiles.append((c, P))
                c += P
            while c < end:
                w = min(FT, end - c)
                tiles.append((c, w))
                c += w
            return tiles

        def _trsm_one(cx, c0, w, tix):
            i, m = cx["i"], cx["m"]
            if m == 1 and i == 0 and tix > 0:
                rhs = gs[0][:, ds(c0, w)]
            elif tix == 0:
                rhs = cx["sb"]
            else:
                rhs = cx["strip"][:, ds(c0 - i * P, w)]
            tp = wpsum.tile([P, FT], f32, tag="w", name="tp")
            nc.tensor.matmul(tp[:, :w], cx["what"], rhs, start=True, stop=True)
            if tix < 2:
                # head tiles gate the next round's diag-ap: DVE copy is
                # faster than scalar for 128-wide and DVE is idle here
                nc.vector.tensor_copy(ubig[m][:, i, ds(c0, w)], tp[:, :w])
            else:
                nc.scalar.copy(ubig[m][:, i, ds(c0, w)], tp[:, :w])

        def emit_trsm_head(cx):
            for tix, (c0, w) in enumerate(_trsm_tiles(cx)[:2]):
                _trsm_one(cx, c0, w, tix)

        def trsm_rest_gen(cx):
            for tix, (c0, w) in enumerate(_trsm_tiles(cx)[2:], start=2):
                _trsm_one(cx, c0, w, tix)
                yield

        # ---- emission schedule ----
        # Panel groups: B0..B6 solo, then (B_{7+i}, A_i) zipped.
        groups = [[(1, i)] for i in range(7)] + [
            [(1, 7 + i), (0, i)] for i in range(NTA)
        ]
        # Gram strip generators drained in order; strip i must complete
        # before any panel with index i starts (both facts share strip i).
        gram_gens = [gram_chunks(i) for i in range(NTB)]
        gram_done = 0  # strips fully drained

        def pull_gram_chunk(limit):
            """Emit one chunk from the next unfinished strip <= limit."""
            nonlocal gram_done
            while gram_done < NTB and gram_done <= limit:
                try:
                    next(gram_gens[gram_done])
                    return True
                except StopIteration:
                    gram_done += 1
            return False

        def drain_gram_through(idx):
            while pull_gram_chunk(idx):
                pass

        def gram_filler(limit):
            """Bounded prefetch: strips beyond `limit` are saved so the late
            (small-trailing) panel rounds still have PE filler."""
            while pull_gram_chunk(limit):
                yield

        def rest_chunks(cx):
            for tix in range(1, len(cx["tiles"])):
                emit_accum_prep(cx["i"], cx["m"], cx, tix)
                yield

        def chain_gens(*gens):
            for g in gens:
                if g is not None:
                    yield from g

        drain_gram_through(0)
        # per-matrix work deferred from the previous round: the TRSM tail
        # (wide MMs -- prime PE filler) then that panel's trace series.
        # Ordering matters: a panel's off-diag Schur preps read the FULL U of
        # the previous panel, so trsm_rest must precede rest_chunks within
        # each matrix's chained generator.
        deferred = {0: None, 1: None}
        deferred_tr = {0: None, 1: None}
        for panels in groups:
            max_strip = max(i for (m, i) in panels)
            drain_gram_through(max_strip)
            gfill = gram_filler(min(max_strip + 2, NTB - 1))
            cxs = [new_panel(i, m) for (m, i) in panels]
            # phase a: PE ap-chains; phase b: prev round's deferred traces
            # (DVE/gpsimd) overlap them; phase c: diag DVE finish.
            for cx in cxs:
                emit_diag_ap(cx)
            tr_prev = [deferred_tr.pop(cx["m"], None) for cx in cxs]
            for g in tr_prev:
                if g is not None:
                    for _ in g:
                        pass
            for cx in cxs:
                emit_diag_fin(cx)
            fillers = []
            for cx in cxs:
                fillers.append(
                    chain_gens(deferred.pop(cx["m"], None), rest_chunks(cx))
                )
            fillers.append(gfill)
            gens = [refine_gen(cx) for cx in cxs]
            live = list(gens)
            fi = 0
            while live:
                for g in list(live):
                    try:
                        next(g)
                    except StopIteration:
                        live.remove(g)
                # one filler chunk between refine steps
                while fillers:
                    f = fillers[fi % len(fillers)]
                    try:
                        next(f)
                        break
                    except StopIteration:
                        fillers.remove(f)
                fi += 1
            # drain remaining non-gram fillers (gfill spans groups)
            for f in fillers:
                if f is not gfill:
                    for _ in f:
                        pass
            for cx in cxs:
                emit_trsm_head(cx)
                deferred[cx["m"]] = trsm_rest_gen(cx)
                deferred_tr[cx["m"]] = trace_chunks(cx)
        # flush the last panels' deferred TRSM tails; the finale's Ln pass
        # (below) overlaps the final trace series on the ACT engine.
        for m in (0, 1):
            for g in (deferred.get(m),):
                if g is not None:
                    for _ in g:
                        pass

        # -------- final: batched Ln(d), partition-sum via matmul ------
        lnall = vpool.tile([P, NPAN], f32, tag="lnall", name="lnall")
        nc.scalar.activation(lnall, dstore, AF.Ln)
        ln0 = vpool.tile([P, 1], f32, tag="ln0", name="ln0")
        nc.vector.tensor_reduce(ln0, lnall[:, 0:NTA], AX, OP.add)
        ln1 = vpool.tile([P, 1], f32, tag="ln1", name="ln1")
        nc.vector.tensor_reduce(ln1, lnall[:, NTA:NPAN], AX, OP.add)
        # last panels' trace series: emitted after the Ln kickoff so the ACT
        # table load + Ln overlap these DVE/gpsimd ops
        for m in (0, 1):
            g = deferred_tr.get(m)
            if g is not None:
                for _ in g:
                    pass
        accd = vpool.tile([P, 1], f32, tag="accd", name="accd")
        nc.vector.tensor_sub(accd, acc[:, 0:1], acc[:, 1:2])
        nc.vector.tensor_add(accd, accd, ln0)
        nc.vector.tensor_sub(accd, accd, ln1)
        ones = vpool.tile([P, 1], f32, tag="ones", name="ones")
        nc.vector.memset(ones, 1.0)
        r_ps = wpsum.tile([P, FT], f32, tag="w", name="r_ps")
        nc.tensor.matmul(r_ps[:1, :1], accd, ones, start=True, stop=True)
        res = vpool.tile([1, 1], f32, tag="res", name="res")
        nc.vector.tensor_copy(res, r_ps[:1, :1])
        nc.sync.dma_start(out_d[:, :], res)

    nc.finalize()
    return nc


FP8_SCALE = 16.0  # B fed as fp8_e4m3 * 16 -> Gram = 256*G; logdet fixed below
OUT_FIX = (N - NS) * np.log(FP8_SCALE * FP8_SCALE)


def make_in_maps(x, B):
    """Host-side prep: per-core column-permuted B (selected first) + masks."""
    bs, n = x.shape
    k = B.shape[0]
    bpad = np.zeros((N, N), dtype=ml_dtypes.float8_e4m3)
    bpad[:k, :] = (B * FP8_SCALE).astype(ml_dtypes.float8_e4m3)
    in_maps = []
    for c in range(bs):
        selmask = x[c] == 1
        nsel = int(selmask.sum())
        assert nsel <= NS, f"sample {c}: nsel={nsel} > window {NS}"
        perm = np.concatenate([np.where(selmask)[0], np.where(~selmask)[0]])
        m = (np.arange(NS) < nsel).astype(np.float32)
        in_maps.append({
            "bb": np.ascontiguousarray(bpad[:, perm]),
            "mrow": m.astype(ml_dtypes.bfloat16).reshape(1, NS),
            "mcol": m.reshape(NS, 1),
        })
    return in_maps


def kernel(x, B):
    """Full inputs -> full output. x: [8, 2048] int32, B: [2000, 2048] f32."""
    from concourse.bass_utils import run_bass_kernel_spmd

    bs, n = x.shape
    assert n == N and bs == 8

    if "nc" not in _CACHE:
        _CACHE["nc"] = _build()
    nc = _CACHE["nc"]

    in_maps = make_in_maps(x, B)
    res = run_bass_kernel_spmd(nc, in_maps, core_ids=list(range(bs)))
    out = np.array(
        [r["out"][0, 0] + OUT_FIX for r in res.results], dtype=np.float32
    )
    return out


# revision 24
# speedup vs baseline: 1.0368x; 1.0368x over previous
"""Trainium2 Bass kernel for nn_DPP: batched masked-Gram logdet minus shared
normalizer logdet.

out[i] = logdet(G * m_i m_i^T + diag(1-m_i)) - logdet(G + I),  G = B^T B

Sharding: data-parallel over the batch dim of x (one sample per NeuronCore).
Host-side trick: each core receives B with its sample's SELECTED columns
permuted to the front.  Then ONE Gram G' = Bperm^T Bperm serves both
factorizations:
  - masked matrix = leading [1152 x 1152] block of G' with a contiguous
    prefix mask (nsel <= 1058 < 1152 for this problem) -> 9-panel Cholesky
    instead of 16 (the trailing 896+ masked cols are identity rows, det 1).
  - normalizer  = G' + I (full 2048, det invariant under permutation)
    -> 16-panel Cholesky.
Each core computes the shared logdet(G+I) redundantly (no cross-core
traffic; collectives here cost more than the 4.5 MB recompute).

Device algorithm (per core):
  - B fed as fp8_e4m3 (x16); G' strips via double-pumped fp8 matmuls
    (DoubleRow, 256-deep contraction, fp32 PSUM accum) -> Gram = 256*G',
    diag fixes scaled by 256, host adds (N-NS)*ln(256) back.
  - Two interleaved left-looking blocked Cholesky factorizations (U-form,
    128-wide panels): A = leading window masked (9 panels), B = G'+I (16
    panels).  B panels 0-6 run solo first (their big Schur updates + gram
    strips are PE filler), then (B_{7+i}, A_i) zip so both refine chains
    overlap; trailing widths shrink together.
  - U panels stored fp8 in per-matrix tensors with uniform panel stride so
    Schur accumulations pair 2 panels per DoubleRow matmul.
  - Each 128x128 diagonal pivot S is handled matmul-only ("refine" scheme):
      d = diag(S); r = 1/sqrt(d); c1 = S * (r r^T); X1 = striu(c1)
      W = diag(r) (I - X1);  F = (I-X1)^T c1 (I-X1) - I
      logdet(S) = sum(ln d) + tr F - tr F^2/2
      What = W (I - F/2)                          (What What^T ~ S^{-1})
    Panel: U_strip = What^T @ strip; trailing Schur updates use U (fp8).
    Latency hiding: trace series + TRSM tails defer into the next round;
    all ln d batch into one ACT Ln at the end.
"""

import numpy as np
import ml_dtypes

P = 128
N = 2048           # full matrix dim (= n columns of B)
NTB = 16           # panels of the normalizer factorization
NTA = 9            # panels of the masked factorization (window 1152)
NS = NTA * P       # masked window = 1152 cols
NKT = 16           # contraction tiles (B rows padded 2000 -> 2048)
FT = 512           # free-dim tile for wide matmuls

_CACHE = {}


def _col_tiles(width_blocks, base_col, diag_first=False):
    """Split absolute cols [base_col, base_col + width_blocks*128) into <=512
    tiles. With diag_first, the first tile is exactly 128 wide (diag block)."""
    tiles = []
    c = base_col
    end = base_col + width_blocks * P
    if diag_first:
        tiles.append((c, P))
        c += P
    while c < end:
        w = min(FT, end - c)
        tiles.append((c, w))
        c += w
    return tiles


def _build():
    import concourse.bass as bass
    import concourse.bacc as bacc
    import concourse.mybir as mybir
    from concourse.bass import ds, ts
    from concourse.masks import (
        make_identity,
        make_upper_triangular,
        make_lower_triangular,
    )
    from concourse.tile import TileContext
    from contextlib import ExitStack

    f32 = mybir.dt.float32
    bf16 = mybir.dt.bfloat16
    f8 = mybir.dt.float8e4
    DR = mybir.MatmulPerfMode.DoubleRow
    AF = mybir.ActivationFunctionType
    OP = mybir.AluOpType
    PSUM = bass.MemorySpace.PSUM
    AX = mybir.AxisListType.X

    NPAN = NTA + NTB  # 25 total panels

    nc = bacc.Bacc()
    bb = nc.dram_tensor("bb", [N, N], f8, kind="ExternalInput")
    mrow_d = nc.dram_tensor("mrow", [1, NS], bf16, kind="ExternalInput")
    mcol_d = nc.dram_tensor("mcol", [NS, 1], f32, kind="ExternalInput")
    out_d = nc.dram_tensor("out", [1, 1], f32, kind="ExternalOutput")

    with TileContext(nc) as tc, ExitStack() as stack:
        consts = stack.enter_context(tc.tile_pool(name="consts", bufs=1))
        I128 = consts.tile([P, P], f32, tag="i128")
        make_identity(nc, I128)
        I128b = consts.tile([P, P], bf16, tag="i128b")
        nc.vector.tensor_copy(I128b, I128)
        STRIU = consts.tile([P, P], f32, tag="striu")
        make_upper_triangular(nc, STRIU, val=1.0, diag=False)
        STRIUB = consts.tile([P, P], bf16, tag="striub")
        nc.vector.tensor_copy(STRIUB, STRIU)
        mrow = consts.tile([1, NS], bf16, tag="mrow")
        nc.sync.dma_start(mrow, mrow_d[:, :])
        mcol = consts.tile([P, NTA], f32, tag="mcol")
        nc.sync.dma_start(mcol, mcol_d.rearrange("(t p) one -> p (t one)", p=P))
        acc = consts.tile([P, 2], f32, tag="acc")
        nc.vector.memset(acc, 0.0)
        dstore = consts.tile([P, NPAN], f32, tag="dstore")
        # B is fed as fp8 scaled by 16, so the Gram is 256*G; diag fixes are
        # scaled by 256 to match and the host adds (N-NS)*ln(256) back.
        SC = 256.0
        onem_all = consts.tile([P, NTA], f32, tag="onem_all")
        nc.vector.tensor_scalar(
            out=onem_all, in0=mcol, scalar1=-SC, scalar2=SC,
            op0=OP.mult, op1=OP.add,
        )
        I256 = consts.tile([P, P], f32, tag="i256")
        nc.vector.tensor_scalar(
            out=I256, in0=I128, scalar1=SC, scalar2=None, op0=OP.mult
        )
        # diag fix for masked panels: SC*diag(1-m) per 128-block
        dfix_all = consts.tile([P, NTA, P], f32, tag="dfix_all")
        for i in range(NTA):
            nc.vector.tensor_scalar_mul(dfix_all[:, i, :], I128, onem_all[:, ds(i, 1)])

        # shared Gram strips: gs[i]: [P, (NTB-i)*P] bf16, cols i*128..2048
        gs = []
        for i in range(NTB):
            gs.append(consts.tile([P, (NTB - i) * P], bf16, tag=f"gs{i}", name=f"gs{i}"))
        # U panels, fp8, one tensor per factorization with ABSOLUTE columns:
        # ubig[m][:, j, c] = U_j[:, c].  Uniform panel stride lets the Schur
        # chains pair two panels into one DoubleRow (double-pumped) matmul.
        # fp8 U storage costs ~0.1 abs logdet error (CPU-simulated; budget 30).
        ubig = {
            0: consts.tile([P, NTA, NTA * P], f8, tag="ubigA", name="ubigA"),
            1: consts.tile([P, NTB, NTB * P], f8, tag="ubigB", name="ubigB"),
        }

        NT_of = {0: NTA, 1: NTB}

        bpool = stack.enter_context(tc.tile_pool(name="bpool", bufs=1))
        gpsum = stack.enter_context(tc.tile_pool(name="gram_psum", bufs=2, space=PSUM))
        spool = stack.enter_context(tc.tile_pool(name="strip_pool", bufs=2))
        rpool = stack.enter_context(tc.tile_pool(name="ref_pool", bufs=2))
        vpool = stack.enter_context(tc.tile_pool(name="vec_pool", bufs=2))
        apsum = stack.enter_context(tc.tile_pool(name="acc_psum", bufs=2, space=PSUM))
        wpsum = stack.enter_context(tc.tile_pool(name="work_psum", bufs=4, space=PSUM))

        bt = bpool.tile([P, NKT, N], f8, tag="bt")
        # per-ktile DMAs so the first Gram chains can start before the full
        # 4.2 MB lands (a single DMA serialized ~35 us of startup)
        for kt in range(NKT):
            nc.sync.dma_start(bt[:, kt, :], bb[ds(kt * P, P), :])

        def gram_chunks(i):
            """One yield per <=512-wide tile of Gram strip i (8 double-pumped
            fp8 MMs, 256-deep contraction each)."""
            for (c0, w) in _col_tiles(NTB - i, i * P):
                pt = gpsum.tile([P, FT], f32, tag="gp", name="pt")
                for kt in range(NKT // 2):
                    nc.tensor.matmul(
                        pt[:, :w],
                        bt[:, ds(2 * kt, 2), ts(i, P)],
                        bt[:, ds(2 * kt, 2), ds(c0, w)],
                        start=(kt == 0),
                        stop=(kt == NKT // 2 - 1),
                        perf_mode=DR,
                    )
                nc.scalar.copy(gs[i][:, ds(c0 - i * P, w)], pt[:, :w])
                yield

        def new_panel(i, m):
            wblk = NT_of[m] - i
            return {
                "m": m,
                "i": i,
                "tiles": _col_tiles(wblk, i * P, diag_first=True),
                # tiles that outlive the group (read by deferred TRSM/traces
                # emitted during the NEXT group) get per-matrix tags so the
                # bufs=2 rotation can't clobber them early.
                "strip": spool.tile(
                    [P, wblk * P], bf16, tag=f"strip{m}", name="strip"
                ),
                "sblk": rpool.tile([P, P], f32, tag="sblk", name="sblk"),
                "sb": rpool.tile([P, P], bf16, tag="sb", name="sb"),
                # dstore column and acc column for this panel
                "dcol": i if m == 0 else NTA + i,
                "acol": m,
            }

        def emit_ap_chain(m, i, c0, w):
            """Schur accumulator for cols [c0, c0+w): pairs of fp8 U panels
            via double-pumped matmuls."""
            ap = apsum.tile([P, FT], f32, tag="ap", name="ap")
            npair = i // 2
            for jp in range(npair):
                nc.tensor.matmul(
                    ap[:, :w],
                    ubig[m][:, ds(2 * jp, 2), ds(i * P, P)],
                    ubig[m][:, ds(2 * jp, 2), ds(c0, w)],
                    start=(jp == 0),
                    stop=(jp == npair - 1 and i % 2 == 0),
                    perf_mode=DR,
                )
            if i % 2 == 1:
                nc.tensor.matmul(
                    ap[:, :w],
                    ubig[m][:, i - 1, ds(i * P, P)],
                    ubig[m][:, i - 1, ds(c0, w)],
                    start=(i == 1),
                    stop=True,
                )
            return ap

        def emit_diag_ap(cx):
            """PE phase of the diag-tile prep (emitted first so deferred
            trace/DVE work can overlap it on other engines)."""
            i, m = cx["i"], cx["m"]
            if i > 0:
                cx["diag_ap"] = emit_ap_chain(m, i, i * P, P)

        def emit_diag_fin(cx):
            """DVE phase of the diag-tile prep."""
            i, m = cx["i"], cx["m"]
            sblk, sb = cx["sblk"], cx["sb"]
            ap = cx.get("diag_ap")
            gsl = gs[i][:, ds(0, P)]
            if m == 0:
                mo = wpsum.tile([P, FT], f32, tag="w", name="mo")
                nc.tensor.matmul(
                    mo[:, :P], mrow[:, ts(i, P)], mrow[:, ts(i, P)],
                    start=True, stop=True,
                )
                tmp = rpool.tile([P, P], f32, tag="tmp", name="tmp")
                nc.vector.tensor_mul(tmp, gsl, mo[:, :P])
                if i > 0:
                    tmp2 = rpool.tile([P, P], f32, tag="tmp2", name="tmp2")
                    nc.vector.tensor_sub(tmp2, tmp, ap[:, :P])
                else:
                    tmp2 = tmp
                nc.vector.tensor_add(sblk, tmp2, dfix_all[:, i, :])
            else:
                if i > 0:
                    tmp = rpool.tile([P, P], f32, tag="tmp", name="tmp")
                    nc.vector.tensor_sub(tmp, gsl, ap[:, :P])
                    nc.vector.tensor_add(sblk, tmp, I256)
                else:
                    nc.vector.tensor_add(sblk, gsl, I256)
            nc.vector.tensor_copy(sb, sblk)

        def emit_accum_prep(i, m, cx, tix):
            """Accum psum chain + strip-prep for OFF-DIAG tile tix (>0)."""
            c0, w = cx["tiles"][tix]
            strip = cx["strip"]
            ap = None
            if i > 0:
                ap = emit_ap_chain(m, i, c0, w)
            gsl = gs[i][:, ds(c0 - i * P, w)]
            if m == 0:
                # masked window: strip = gs * (m m^T) [- ap]
                mo = wpsum.tile([P, FT], f32, tag="w", name="mo")
                nc.tensor.matmul(
                    mo[:, :w], mrow[:, ts(i, P)], mrow[:, ds(c0, w)],
                    start=True, stop=True,
                )
                tmp3 = spool.tile([P, FT], f32, tag="ptmp", name="tmp3")
                nc.vector.tensor_mul(tmp3[:, :w], gsl, mo[:, :w])
                if i > 0:
                    nc.vector.tensor_sub(
                        strip[:, ds(c0 - i * P, w)], tmp3[:, :w], ap[:, :w]
                    )
                else:
                    nc.vector.tensor_copy(
                        strip[:, ds(c0 - i * P, w)], tmp3[:, :w]
                    )
            else:
                if i > 0:
                    nc.vector.tensor_sub(
                        strip[:, ds(c0 - i * P, w)], gsl, ap[:, :w]
                    )
                # (m=1, i=0): TRSM reads gs[0] directly

        def refine_gen(cx):
            """Pivot-block factor; yields at cross-engine handoffs so filler
            matmuls can be emitted between dependent steps.  W = diag(r)(I-X1)
            first-order with one -F/2 refinement; F+I = W'^T c1 W' is computed
            straight from the normalized pivot c1 (c1 = diag(r) S diag(r)), so
            the W-scaling (wfac) and its transpose (wt) hang OFF the critical
            chain.  Trace series (to F^2) deferred into trace_chunks()."""
            m = cx["m"]
            sblk = cx["sblk"]
            dcol = dstore[:, ds(cx["dcol"], 1)]
            dummy = rpool.tile([P, P], f32, tag="dummy", name="dummy")
            nc.vector.tensor_mul(dummy, sblk, I128)
            nc.vector.tensor_reduce(dcol, dummy, AX, OP.add)
            rinv = vpool.tile([P, 1], f32, tag="rinv", name="rinv")
            nc.vector.reciprocal(rinv, dcol)
            rcol = vpool.tile([P, 1], f32, tag="rcol", name="rcol")
            nc.scalar.sqrt(rcol, rinv)
            yield
            rt_ps = wpsum.tile([P, FT], f32, tag="w", name="rt_ps")
            nc.tensor.transpose(rt_ps[:1, :P], rcol, I128)
            rrow = vpool.tile([1, P], bf16, tag="rrow", name="rrow")
            nc.vector.tensor_copy(rrow, rt_ps[:1, :P])
            yield
            q_ps = wpsum.tile([P, FT], f32, tag="w", name="q_ps")
            nc.tensor.matmul(q_ps[:, :P], rrow, rrow, start=True, stop=True)
            c1b = rpool.tile([P, P], bf16, tag="c1b", name="c1b")
            nc.vector.tensor_mul(c1b, sblk, q_ps[:, :P])
            yield
            x1 = rpool.tile([P, P], bf16, tag="x1", name="x1")
            nc.gpsimd.tensor_mul(x1, c1b, STRIUB)
            wser = rpool.tile([P, P], bf16, tag="wser", name="wser")
            nc.vector.tensor_sub(wser, I128b, x1)
            yield
            y_ps = wpsum.tile([P, FT], f32, tag="w", name="y_ps")
            nc.tensor.matmul(y_ps[:, :P], c1b, wser, start=True, stop=True)
            yb = rpool.tile([P, P], bf16, tag="yb", name="yb")
            nc.vector.tensor_copy(yb, y_ps[:, :P])
            # off-chain: wfac = diag(r) (I - X1)
            wfac = rpool.tile([P, P], bf16, tag="wfac", name="wfac")
            nc.vector.tensor_scalar_mul(wfac, wser, rcol)
            yield
            f_ps = wpsum.tile([P, FT], f32, tag="w", name="f_ps")
            nc.tensor.matmul(f_ps[:, :P], wser, yb, start=True, stop=True)
            ff = rpool.tile([P, P], bf16, tag=f"ff{m}", name="ff")
            nc.vector.tensor_sub(ff, f_ps[:, :P], I128)
            fs = rpool.tile([P, P], bf16, tag="fs", name="fs")
            nc.vector.tensor_scalar_mul(fs, ff, -0.5)
            yield
            # off-chain: wt = wfac^T (ready before wh thanks to the F chain)
            wt_ps = wpsum.tile([P, FT * 2], bf16, tag="w", name="wt_ps")
            nc.tensor.transpose(wt_ps[:, :P], wfac, I128b)
            wt = rpool.tile([P, P], bf16, tag="wt", name="wt")
            nc.vector.tensor_copy(wt, wt_ps[:, :P])
            yield
            wh_ps = wpsum.tile([P, FT], f32, tag="w", name="wh_ps")
            nc.tensor.matmul(wh_ps[:, :P], wt, fs, start=True, stop=True)
            what = rpool.tile([P, P], bf16, tag=f"what{m}", name="what")
            nc.vector.tensor_add(what, wh_ps[:, :P], wfac)
            cx["what"] = what
            cx["ff"] = ff

        def trace_chunks(cx):
            """Deferred logdet trace series (to F^2) for a finished panel:
            emitted as filler in the NEXT round so it never sits in an engine
            queue ahead of the refine chain's dependent ops."""
            ff = cx["ff"]
            dummy3 = rpool.tile([P, P], f32, tag="dummy3", name="dummy3")
            nc.gpsimd.tensor_mul(dummy3, ff, I128)
            trf = vpool.tile([P, 1], f32, tag="trf", name="trf")
            nc.vector.tensor_reduce(trf, dummy3, AX, OP.add)
            yield
            dummy4 = rpool.tile([P, P], f32, tag="dummy4", name="dummy4")
            nc.gpsimd.tensor_mul(dummy4, ff, ff)
            trf2 = vpool.tile([P, 1], f32, tag="trf2", name="trf2")
            nc.vector.tensor_reduce(trf2, dummy4, AX, OP.add)
            yield
            t1 = vpool.tile([P, 1], f32, tag="t1", name="t1")
            t2 = vpool.tile([P, 1], f32, tag="t2", name="t2")
            nc.vector.tensor_scalar(
                out=t2, in0=trf2, scalar1=-0.5, scalar2=None, op0=OP.mult
            )
            nc.vector.tensor_add(t1, trf, t2)
            ac = cx["acol"]
            nc.vector.tensor_add(acc[:, ds(ac, 1)], acc[:, ds(ac, 1)], t1)

        def _trsm_tiles(cx):
            """TRSM tiling: diag, one 128 block, then <=512 chunks.  The
            first two are emitted in-round (the next diag-prep needs U's
            col-block 1); the rest defers into the next round as PE filler."""
            i, m = cx["i"], cx["m"]
            base, end = i * P, NT_of[m] * P
            tiles = [(base, P)]
            c = base + P
            if c < end:
                tiles.append((c, P))
                c += P
            while c < end:
                w = min(FT, end - c)
                tiles.append((c, w))
                c += w
            return tiles

        def _trsm_one(cx, c0, w, tix):
            i, m = cx["i"], cx["m"]
            if m == 1 and i == 0 and tix > 0:
                rhs = gs[0][:, ds(c0, w)]
            elif tix == 0:
                rhs = cx["sb"]
            else:
                rhs = cx["strip"][:, ds(c0 - i * P, w)]
            tp = wpsum.tile([P, FT], f32, tag="w", name="tp")
            nc.tensor.matmul(tp[:, :w], cx["what"], rhs, start=True, stop=True)
            nc.scalar.copy(ubig[m][:, i, ds(c0, w)], tp[:, :w])

        def emit_trsm_head(cx):
            for tix, (c0, w) in enumerate(_trsm_tiles(cx)[:2]):
                _trsm_one(cx, c0, w, tix)

        def trsm_rest_gen(cx):
            for tix, (c0, w) in enumerate(_trsm_tiles(cx)[2:], start=2):
                _trsm_one(cx, c0, w, tix)
                yield

        # ---- emission schedule ----
        # Panel groups: B0..B6 solo, then (B_{7+i}, A_i) zipped.
        groups = [[(1, i)] for i in range(7)] + [
            [(1, 7 + i), (0, i)] for i in range(NTA)
        ]
        # Gram strip generators drained in order; strip i must complete
        # before any panel with index i starts (both facts share strip i).
        gram_gens = [gram_chunks(i) for i in range(NTB)]
        gram_done = 0  # strips fully drained

        def pull_gram_chunk(limit):
            """Emit one chunk from the next unfinished strip <= limit."""
            nonlocal gram_done
            while gram_done < NTB and gram_done <= limit:
                try:
                    next(gram_gens[gram_done])
                    return True
                except StopIteration:
                    gram_done += 1
            return False

        def drain_gram_through(idx):
            while pull_gram_chunk(idx):
                pass

        def gram_filler(limit):
            """Bounded prefetch: strips beyond `limit` are saved so the late
            (small-trailing) panel rounds still have PE filler."""
            while pull_gram_chunk(limit):
                yield

        def rest_chunks(cx):
            for tix in range(1, len(cx["tiles"])):
                emit_accum_prep(cx["i"], cx["m"], cx, tix)
                yield

        def chain_gens(*gens):
            for g in gens:
                if g is not None:
                    yield from g

        drain_gram_through(0)
        # per-matrix work deferred from the previous round: the TRSM tail
        # (wide MMs -- prime PE filler) then that panel's trace series.
        # Ordering matters: a panel's off-diag Schur preps read the FULL U of
        # the previous panel, so trsm_rest must precede rest_chunks within
        # each matrix's chained generator.
        deferred = {0: None, 1: None}
        deferred_tr = {0: None, 1: None}
        for panels in groups:
            max_strip = max(i for (m, i) in panels)
            drain_gram_through(max_strip)
            gfill = gram_filler(min(max_strip + 2, NTB - 1))
            cxs = [new_panel(i, m) for (m, i) in panels]
            # phase a: PE ap-chains; phase b: prev round's deferred traces
            # (DVE/gpsimd) overlap them; phase c: diag DVE finish.
            for cx in cxs:
                emit_diag_ap(cx)
            tr_prev = [deferred_tr.pop(cx["m"], None) for cx in cxs]
            for g in tr_prev:
                if g is not None:
                    for _ in g:
                        pass
            for cx in cxs:
                emit_diag_fin(cx)
            fillers = []
            for cx in cxs:
                fillers.append(
                    chain_gens(deferred.pop(cx["m"], None), rest_chunks(cx))
                )
            fillers.append(gfill)
            gens = [refine_gen(cx) for cx in cxs]
            live = list(gens)
            fi = 0
            while live:
                for g in list(live):
                    try:
                        next(g)
                    except StopIteration:
                        live.remove(g)
                # one filler chunk between refine steps
                while fillers:
                    f = fillers[fi % len(fillers)]
                    try:
                        next(f)
                        break
                    except StopIteration:
                        fillers.remove(f)
                fi += 1
            # drain remaining non-gram fillers (gfill spans groups)
            for f in fillers:
                if f is not gfill:
                    for _ in f:
                        pass
            for cx in cxs:
                emit_trsm_head(cx)
                deferred[cx["m"]] = trsm_rest_gen(cx)
                deferred_tr[cx["m"]] = trace_chunks(cx)
        # flush the last panels' deferred TRSM tails; the finale's Ln pass
        # (below) overlaps the final trace series on the ACT engine.
        for m in (0, 1):
            for g in (deferred.get(m),):
                if g is not None:
                    for _ in g:
                        pass

        # -------- final: batched Ln(d), partition-sum via matmul ------
        lnall = vpool.tile([P, NPAN], f32, tag="lnall", name="lnall")
        nc.scalar.activation(lnall, dstore, AF.Ln)
        ln0 = vpool.tile([P, 1], f32, tag="ln0", name="ln0")
        nc.vector.tensor_reduce(ln0, lnall[:, 0:NTA], AX, OP.add)
        ln1 = vpool.tile([P, 1], f32, tag="ln1", name="ln1")
        nc.vector.tensor_reduce(ln1, lnall[:, NTA:NPAN], AX, OP.add)
        # last panels' trace series: emitted after the Ln kickoff so the ACT
        # table load + Ln overlap these DVE/gpsimd ops
        for m in (0, 1):
            g = deferred_tr.get(m)
            if g is not None:
                for _ in g:
                    pass
        accd = vpool.tile([P, 1], f32, tag="accd", name="accd")
        nc.vector.tensor_sub(accd, acc[:, 0:1], acc[:, 1:2])
        nc.vector.tensor_add(accd, accd, ln0)
        nc.vector.tensor_sub(accd, accd, ln1)
        ones = vpool.tile([P, 1], f32, tag="ones", name="ones")
        nc.vector.memset(ones, 1.0)
        r_ps = wpsum.tile([P, FT], f32, tag="w", name="r_ps")
        nc.tensor.matmul(r_ps[:1, :1], accd, ones, start=True, stop=True)
        res = vpool.tile([1, 1], f32, tag="res", name="res")
        nc.vector.tensor_copy(res, r_ps[:1, :1])
        nc.sync.dma_start(out_d[:, :], res)

    nc.finalize()
    return nc


FP8_SCALE = 16.0  # B fed as fp8_e4m3 * 16 -> Gram = 256*G; logdet fixed below
OUT_FIX = (N - NS) * np.log(FP8_SCALE * FP8_SCALE)


def make_in_maps(x, B):
    """Host-side prep: per-core column-permuted B (selected first) + masks."""
    bs, n = x.shape
    k = B.shape[0]
    bpad = np.zeros((N, N), dtype=ml_dtypes.float8_e4m3)
    bpad[:k, :] = (B * FP8_SCALE).astype(ml_dtypes.float8_e4m3)
    in_maps = []
    for c in range(bs):
        selmask = x[c] == 1
        nsel = int(selmask.sum())
        assert nsel <= NS, f"sample {c}: nsel={nsel} > window {NS}"
        perm = np.concatenate([np.where(selmask)[0], np.where(~selmask)[0]])
        m = (np.arange(NS) < nsel).astype(np.float32)
        in_maps.append({
            "bb": np.ascontiguousarray(bpad[:, perm]),
            "mrow": m.astype(ml_dtypes.bfloat16).reshape(1, NS),
            "mcol": m.reshape(NS, 1),
        })
    return in_maps


def kernel(x, B):
    """Full inputs -> full output. x: [8, 2048] int32, B: [2000, 2048] f32."""
    from concourse.bass_utils import run_bass_kernel_spmd

    bs, n = x.shape
    assert n == N and bs == 8

    if "nc" not in _CACHE:
        _CACHE["nc"] = _build()
    nc = _CACHE["nc"]

    in_maps = make_in_maps(x, B)
    res = run_bass_kernel_spmd(nc, in_maps, core_ids=list(range(bs)))
    out = np.array(
        [r["out"][0, 0] + OUT_FIX for r in res.results], dtype=np.float32
    )
    return out
